# revision 55
# baseline (speedup 1.0000x reference)
"""Trainium2 Bass kernel for nn_Attention_13829794693777.

Multi-head attention (8 heads, head_dim 48) + LePE depthwise 3x3 conv on v.
Sharding: tensor-parallel over heads -- one head per NeuronCore (8 cores).
Each core gets the full (transposed, bf16) input activations plus its head's
qkv weight slice and LePE channel slice; it produces the [seq, 48] channel
slice of the output. The host concatenates slices and reshapes.

Device-side dataflow (per core / head):
  1. Projection (PE): qT/kT channel-major [128(dup), seq] bf16; q weights are
     pre-scaled by SCALE*C1 (C1 = 128/ln2) so raw scores arrive scaled for
     the Schraudolph-exp trick. v goes into a 66x66 zero-padded bf16 image
     (vT_pad). Bias copies q/k run on ACT (activation Identity+bias), the v
     scatter on DVE.
  2. v_aug [seq-tile, 49] bf16 via PE transposes reading the padded image
     directly; psum->SBUF copies batched 4-at-a-time on ACT.
  3. LePE: computed per chunk on PE as 9 shifted transpose-matmuls against
     diagonal tap matrices (+1 ones-row matmul for the bias), accumulating
     [pixel, ch] directly into the chunk's PSUM epilogue region.
  4. Attention per 512-query chunk, pipelined over a 3-deep pool of 2-bank
     PSUM scores tiles: S^T[k, q] matmuls (PE) -> exp (split ACT real-Exp /
     DVE Schraudolph int16-bitcast-bf16) -> PV matmuls out[q, ch] with P as
     stationary (output free dim = 49, not 512).
  5. Epilogue per chunk: reciprocal of the ones-column denominators + fused
     affine (out = attn*rec + lepe^T) on DVE, DMA out seq-major.
"""

import numpy as np
import ml_dtypes

NUM_HEADS = 8
DIM = 384
HD = 48
B = 2
N = 4096
SEQ = B * N          # 8192
IMG = 64             # H = W = 64
PADW = IMG + 2       # 66
PADN = PADW * PADW   # 4356
SCALE = HD ** -0.5
NCHUNK = SEQ // 512  # 16 query chunks of 512
KT_PER_B = N // 128  # 32 k-tiles per batch

C1 = float(128.0 / np.log(2.0))   # Schraudolph scale (baked into wq)
C2 = 16250.5                      # Schraudolph bias (RNE-calibrated)

# exp engine pattern per chunk: batches of 2 k-tiles.
# 'A' = ACT real exp, 'D' = DVE Schraudolph.
EXP_PATTERN = ['A', 'A', 'D', 'A', 'D', 'A', 'D', 'A',
               'D', 'A', 'D', 'A', 'D', 'A', 'D', 'A']
EXP_BW = 2
PV_LAG = 10
# LePE: taps with index in PE_TAPS stay on PE (diag matmuls); the rest are
# computed on DVE channel-major per image-eighth and transpose-accumulated.
PE_TAPS = [0, 4, 8]
LEPE_STEP = 6

_CACHE = {}


def _build_module():
    """Build (once) the Bacc module shared by all 8 cores."""
    import concourse.bacc as bacc
    import concourse.bass as bass
    import concourse.mybir as mybir
    import concourse.tile as tile
    from concourse.dve_ops import AFFINE_THEN_ADD
    from contextlib import ExitStack

    dt = mybir.dt
    AF = mybir.ActivationFunctionType
    ALU = mybir.AluOpType

    nc = bacc.Bacc("TRN2", target_bir_lowering=False, debug=False, num_devices=8)

    # ---- DRAM parameters -------------------------------------------------
    xT_d = nc.dram_tensor("xT", [3, 128, SEQ], dt.bfloat16, kind="ExternalInput").ap()
    wqd_d = nc.dram_tensor("wqd", [3, 128, 128], dt.bfloat16, kind="ExternalInput").ap()
    wkd_d = nc.dram_tensor("wkd", [3, 128, 128], dt.bfloat16, kind="ExternalInput").ap()
    wv_d = nc.dram_tensor("wv", [3, 128, HD], dt.bfloat16, kind="ExternalInput").ap()
    bqd_d = nc.dram_tensor("bqd", [128, 1], dt.float32, kind="ExternalInput").ap()
    bkd_d = nc.dram_tensor("bkd", [128, 1], dt.float32, kind="ExternalInput").ap()
    bvd_d = nc.dram_tensor("bvd", [128, 1], dt.float32, kind="ExternalInput").ap()
    dwt_d = nc.dram_tensor("dwt", [9, 128, HD], dt.bfloat16, kind="ExternalInput").ap()
    lb48_d = nc.dram_tensor("lb48", [128, HD], dt.bfloat16, kind="ExternalInput").ap()
    lw9_d = nc.dram_tensor("lw9", [128, 9], dt.float32, kind="ExternalInput").ap()
    idn_d = nc.dram_tensor("idn", [128, 64], dt.float32, kind="ExternalInput").ap()
    out_d = nc.dram_tensor("out", [64, 128, HD], dt.float32, kind="ExternalOutput").ap()
    out_v = out_d.rearrange("t p c -> p t c")

    with tile.TileContext(nc) as tc, ExitStack() as ctx:
        per = ctx.enter_context(tc.tile_pool(name="per", bufs=1))
        rot = ctx.enter_context(tc.tile_pool(name="rot", bufs=2))
        ptp = ctx.enter_context(tc.tile_pool(name="ptp", bufs=8))

        # ---- persistent SBUF tensors ------------------------------------
        xs = [[per.tile([128, 2048], dt.bfloat16, name=f"x{c}_{j}", tag=f"x{c}_{j}") for j in range(4)]
              for c in range(3)]
        qTd = [per.tile([128, 512], dt.bfloat16, name=f"q{n}", tag=f"q{n}") for n in range(NCHUNK)]
        kTd = [per.tile([128, 512], dt.bfloat16, name=f"k{n}", tag=f"k{n}") for n in range(NCHUNK)]
        v_aug = per.tile([128, 64 * 49], dt.bfloat16, name="vaug", tag="vaug")
        v_aug3 = v_aug[:].rearrange("p (t c) -> p t c", c=49)
        vT_pad = per.tile([128, PADN], dt.bfloat16, name="vpad", tag="vpad")
        vp3 = vT_pad[:].rearrange("p (a b) -> p a b", b=PADW)

        wq_all = per.tile([128, 384], dt.bfloat16, name="wqa", tag="wqa")
        wk_all = per.tile([128, 384], dt.bfloat16, name="wka", tag="wka")
        wv_all = per.tile([128, 3 * HD], dt.bfloat16, name="wva", tag="wva")
        wq_sb = [wq_all[:, 128 * c:128 * c + 128] for c in range(3)]
        wk_sb = [wk_all[:, 128 * c:128 * c + 128] for c in range(3)]
        wv_sb = [wv_all[:, HD * c:HD * c + HD] for c in range(3)]
        bq_sb = per.tile([128, 1], dt.float32, name="bq", tag="bq")
        bk_sb = per.tile([128, 1], dt.float32, name="bk", tag="bk")
        bv_sb = per.tile([128, 1], dt.float32, name="bv", tag="bv")
        acc_e = [per.tile([128, 512], dt.float32, name=f"acc{e}", tag=f"acc{e}")
                 for e in range(8)]
        dw_all = per.tile([128, 9 * HD], dt.bfloat16, name="dwa", tag="dwa")
        dw_sb = [dw_all[:, HD * i:HD * i + HD] for i in range(9)]
        lb_sb = per.tile([128, HD], dt.bfloat16, name="lb48", tag="lb48")
        dw9_sb = per.tile([128, 9], dt.float32, name="dw9", tag="dw9")
        one_sb = per.tile([128, 128], dt.bfloat16, name="ones", tag="ones")
        id_sb = per.tile([128, 64], dt.float32, name="idn", tag="idn")
        idb_sb = per.tile([128, 64], dt.bfloat16, name="idnb", tag="idnb")

        # ---- input DMAs: weights (1 DMA each) on the ACT queue, x split --
        nc.scalar.dma_start(wq_all[:].rearrange("p (c k) -> p c k", c=3),
                            wqd_d.rearrange("c p k -> p c k"))
        nc.scalar.dma_start(wk_all[:].rearrange("p (c k) -> p c k", c=3),
                            wkd_d.rearrange("c p k -> p c k"))
        nc.scalar.dma_start(wv_all[:].rearrange("p (c k) -> p c k", c=3),
                            wv_d.rearrange("c p k -> p c k"))
        nc.scalar.dma_start(bq_sb[:], bqd_d[:])
        nc.scalar.dma_start(bk_sb[:], bkd_d[:])
        nc.scalar.dma_start(bv_sb[:], bvd_d[:])
        for c in range(3):
            eng = nc.sync if c != 1 else nc.scalar
            eng.dma_start(xs[c][0][:, 0:512], xT_d[c, :, 0:512])
        for j in range(4):
            for c in range(3):
                eng = nc.sync if c != 1 else nc.scalar
                if j == 0:
                    eng.dma_start(xs[c][0][:, 512:2048], xT_d[c, :, 512:2048])
                else:
                    eng.dma_start(xs[c][j][:], xT_d[c, :, j * 2048:(j + 1) * 2048])
        nc.scalar.dma_start(dw_all[:].rearrange("p (t c) -> p t c", c=HD),
                            dwt_d.rearrange("t p c -> p t c"))
        nc.scalar.dma_start(lb_sb[:], lb48_d[:])
        nc.scalar.dma_start(dw9_sb[:], lw9_d[:])
        nc.vector.memset(one_sb[0:1, :], 1.0)
        nc.scalar.dma_start(id_sb[:], idn_d[:])
        nc.vector.tensor_copy(idb_sb[:], id_sb[:])

        # zero the padded image (borders must be 0)
        nc.vector.memset(vT_pad[:], 0.0)
        nc.vector.memset(v_aug3[:, :, 48:49], 1.0)

        taps = [(dr, dc) for dr in (-1, 0, 1) for dc in (-1, 0, 1)]
        dve_taps = [i for i in range(9) if i not in PE_TAPS]

        def emit_dve_lepe(e):
            # channel-major taps over image-eighth e (rows 8e..8e+8, both batches)
            def tap(dr, dc):
                return vp3[0:112, 1 + 8 * e + dr:1 + 8 * e + 8 + dr,
                           1 + dc:1 + dc + IMG]
            first = True
            for i in dve_taps:
                dr, dc = taps[i]
                if first:
                    nc.vector.tensor_scalar(acc_e[e][0:112, :], tap(dr, dc),
                                            dw9_sb[0:112, i:i + 1], None,
                                            op0=ALU.mult)
                    first = False
                else:
                    nc.vector.scalar_tensor_tensor(acc_e[e][0:112, :], tap(dr, dc),
                                                   dw9_sb[0:112, i:i + 1],
                                                   acc_e[e][0:112, :],
                                                   op0=ALU.mult, op1=ALU.add)

        # ---- phase 1: projection ----------------------------------------
        psA_ctx = ExitStack()
        psA = psA_ctx.enter_context(tc.tile_pool(name="psA", bufs=2, space="PSUM"))
        for n in range(NCHUNK):
            rhs = [xs[c][n // 4][:, (n % 4) * 512:(n % 4 + 1) * 512] for c in range(3)]
            pq = psA.tile([128, 512], dt.float32, name="pq", tag="pq")
            for c in range(3):
                nc.tensor.matmul(pq[:], wq_sb[c][:], rhs[c],
                                 start=(c == 0), stop=(c == 2))
            nc.scalar.activation(qTd[n][:], pq[:], AF.Identity, bias=bq_sb[:, 0:1])
            pk = psA.tile([128, 512], dt.float32, name="pk", tag="pk")
            for c in range(3):
                nc.tensor.matmul(pk[:], wk_sb[c][:], rhs[c],
                                 start=(c == 0), stop=(c == 2))
            nc.scalar.activation(kTd[n][:], pk[:], AF.Identity, bias=bk_sb[:, 0:1])

            b = n // 8
            rb = 64 * b
            pvv = psA.tile([128, 512], dt.float32, name="pvv", tag="pvv")
            for c in range(3):
                nc.tensor.matmul(pvv[rb:rb + HD, :], wv_sb[c][:], rhs[c],
                                 start=(c == 0), stop=(c == 2),
                                 tile_position=(0, rb))
            # scatter the 512 pixels (8 image rows) into the padded image
            r0 = 8 * (n % 8)
            dest = vp3[rb:rb + HD, 1 + r0:1 + r0 + 8, 1:65]
            nc.vector.tensor_scalar(dest, pvv[rb:rb + HD, :], bv_sb[rb:rb + HD, 0:1],
                                    None, op0=ALU.add)
            if n == 13:
                emit_dve_lepe(0)
            elif n == 14:
                emit_dve_lepe(1)

        psA_ctx.close()

        # ---- phase 2: v_aug via PE transposes from the padded image -----
        ps2_ctx = ExitStack()
        ps2 = ps2_ctx.enter_context(tc.tile_pool(name="ps2", bufs=6, space="PSUM"))
        for g in range(16):
            tq = ps2.tile([128, 192], dt.bfloat16, name="tq", tag="tq")
            for j in range(4):
                t = 4 * g + j
                b = t // 32
                rb = 64 * b
                tt = t % 32
                for h in range(2):
                    nc.tensor.matmul(tq[64 * h:64 * h + 64, 48 * j:48 * j + 48],
                                     vp3[rb:rb + HD, 1 + 2 * tt + h, 1:65],
                                     idb_sb[rb:rb + HD, 0:HD],
                                     is_transpose=True, tile_position=(rb, 64 * h))
            tq3 = tq[:].rearrange("p (t c) -> p t c", c=48)
            if g % 3 != 2:
                nc.scalar.activation(v_aug3[:, 4 * g:4 * g + 4, 0:48], tq3[:],
                                     AF.Copy)
            else:
                nc.vector.tensor_copy(v_aug3[:, 4 * g:4 * g + 4, 0:48], tq3[:])
        ps2_ctx.close()

        # ---- phase 4: main attention loop -------------------------------
        stp = ctx.enter_context(tc.tile_pool(name="stp", bufs=3, space="PSUM"))
        psv = ctx.enter_context(tc.tile_pool(name="psv", bufs=2, space="PSUM"))

        # batch layout: slot-aligned batches of EXP_BW k-tiles (slot = kt % 6)
        nb = KT_PER_B // EXP_BW
        batches = [(EXP_BW * i, EXP_BW * i + EXP_BW) for i in range(nb)]

        for cc in range(NCHUNK):
            bc = cc // 8
            rb = 64 * bc
            if cc < 6:
                emit_dve_lepe(cc + 2)
            pvb = psv.tile([128, 512], dt.float32, name="pvb", tag="pvb")
            pv3 = pvb[:, 0:196].rearrange("p (t c) -> p t c", c=49)

            # LePE [pixel, ch] for the 4 query tiles (cols 256..448):
            # 9 shifted diag-matmuls + 1 ones-row bias matmul, PSUM-accumulated
            # (emitted via emit_lepe after the first scores of the chunk so a
            # pvb-buffer wait can't head-of-line-block the scores stream)
            def emit_lepe(qs):
                # transpose of the DVE-computed taps comes FIRST: qs==0 carries
                # the bank 'start' (zeroes the whole pvb bank); everything else
                # (PE taps, bias, PV) accumulates after it.
                tt = (4 * cc + qs) % 32
                e8 = tt // 4
                nc.tensor.matmul(pvb[:, 256 + 48 * qs:256 + 48 * qs + 48],
                                 acc_e[e8][rb:rb + HD,
                                           128 * (tt % 4):128 * (tt % 4) + 128],
                                 id_sb[rb:rb + HD, 0:HD],
                                 is_transpose=True, tile_position=(rb, 0),
                                 start=(qs == 0), stop=False,
                                 skip_group_check=True)

            def emit_lepe_taps(qs):
                tt = (4 * cc + qs) % 32
                for h in range(2):
                    dst = pvb[64 * h:64 * h + 64, 256 + 48 * qs:256 + 48 * qs + 48]
                    for ii, i in enumerate(PE_TAPS):
                        dr, dc = taps[i]
                        nc.tensor.matmul(dst,
                                         vp3[rb:rb + HD, 1 + 2 * tt + h + dr,
                                             1 + dc:1 + dc + IMG],
                                         dw_sb[i][rb:rb + HD, :],
                                         start=False, stop=False,
                                         tile_position=(rb, 64 * h),
                                         skip_group_check=True)
                    nc.tensor.matmul(dst, one_sb[0:1, 0:64], lb_sb[0:1, :],
                                     start=False, stop=False,
                                     tile_position=(0, 64 * h),
                                     skip_group_check=True)

            pt_of_batch = {}
            bi = 0  # next batch whose scores are fully issued
            st_of_batch = {}

            def emit_exp(bidx):
                w = 512 * EXP_BW
                st = st_of_batch.pop(bidx)
                pt = ptp.tile([128, 512 * EXP_BW], dt.bfloat16, name="pt", tag="pt")
                if EXP_PATTERN[bidx] == 'A':
                    nc.scalar.activation(pt[:, 0:w], st[:, 0:w],
                                         AF.Exp, scale=float(1.0 / C1))
                else:
                    nc.vector.tensor_scalar(
                        pt[:, 0:w].bitcast(mybir.dt.int16),
                        st[:, 0:w], C2, None, op0=ALU.add)
                pt_of_batch[bidx] = pt

            def emit_pv(kt):
                bidx = kt // EXP_BW
                a, _ = batches[bidx]
                pt = pt_of_batch[bidx]
                for qb in range(4):
                    nc.tensor.matmul(pvb[:, 49 * qb:49 * qb + 49],
                                     pt[:, 512 * (kt - a) + 128 * qb:
                                         512 * (kt - a) + 128 * qb + 128],
                                     v_aug3[:, bc * 32 + kt, 0:49],
                                     start=False,
                                     stop=(kt == KT_PER_B - 1 and qb == 3),
                                     tile_position=(0, 0), skip_group_check=True)

            for step in range(KT_PER_B + PV_LAG):
                if step == LEPE_STEP:
                    for qs in range(4):
                        emit_lepe(qs)
                    for qs in range(4):
                        emit_lepe_taps(qs)
                kt = step
                if kt < KT_PER_B:
                    bidx = kt // EXP_BW
                    if kt % EXP_BW == 0:
                        st_of_batch[bidx] = stp.tile([128, 512 * EXP_BW],
                                                     dt.float32, name="st", tag="st")
                    row = 64 * (kt & 1)
                    ktile = kTd[bc * 8 + kt // 4]
                    koff = (kt % 4) * 128
                    j = kt % EXP_BW
                    nc.tensor.matmul(st_of_batch[bidx][:, 512 * j:512 * j + 512],
                                     ktile[row:row + HD, koff:koff + 128],
                                     qTd[cc][row:row + HD, :],
                                     tile_position=(row, 0))
                    if bi < len(batches) and kt + 1 == batches[bi][1]:
                        emit_exp(bi)
                        bi += 1
                pvkt = step - PV_LAG
                if 0 <= pvkt < KT_PER_B:
                    emit_pv(pvkt)

            # ---- epilogue for this 512-query chunk ----------------------
            tmp = rot.tile([128, 196], dt.float32, name="tmp", tag="tmp")
            rec = rot.tile([128, 4], dt.float32, name="rec", tag="rec")
            ot = rot.tile([128, 192], dt.float32, name="ot", tag="ot")
            nc.vector.tensor_copy(tmp[:], pvb[:, 0:196])
            tmp3 = tmp[:].rearrange("p (t c) -> p t c", c=49)
            nc.vector.reciprocal(rec[:], tmp3[:, :, 48:49])
            for qs in range(4):
                nc.vector._custom_dve(AFFINE_THEN_ADD,
                                      out=ot[:, qs * 48:(qs + 1) * 48],
                                      in0=tmp[:, qs * 49:qs * 49 + 48],
                                      in1=pvb[:, 256 + 48 * qs:256 + 48 * qs + 48],
                                      s0=rec[:, qs:qs + 1], s1=0.0)
            nc.sync.dma_start(out_v[:, 4 * cc:4 * cc + 4, :],
                              ot[:].rearrange("p (t c) -> p t c", c=HD))

    nc.compile()
    return nc


def _prep_in_maps(x, qkv_w, qkv_b, lepe_w, lepe_b):
    bf16 = ml_dtypes.bfloat16
    X = np.asarray(x, dtype=np.float32).reshape(SEQ, DIM)
    xT = np.ascontiguousarray(X.T).astype(bf16).reshape(3, 128, SEQ)

    qkv_w = np.asarray(qkv_w, dtype=np.float32)
    qkv_b = np.asarray(qkv_b, dtype=np.float32)
    lepe_w = np.asarray(lepe_w, dtype=np.float32)
    lepe_b = np.asarray(lepe_b, dtype=np.float32)

    idn = np.zeros((128, 64), dtype=np.float32)
    idn[0:64, 0:64] = np.eye(64, dtype=np.float32)
    idn[64:128, 0:64] = np.eye(64, dtype=np.float32)

    qs = SCALE * C1  # bake the Schraudolph scale into the q projection

    in_maps = []
    for h in range(NUM_HEADS):
        sl = slice(h * HD, (h + 1) * HD)
        wq = qkv_w[sl, :] * qs                       # [48, 384]
        wk = qkv_w[DIM + h * HD:DIM + (h + 1) * HD, :]
        wv = qkv_w[2 * DIM + h * HD:2 * DIM + (h + 1) * HD, :]
        wqd = np.zeros((3, 128, 128), dtype=np.float32)
        wkd = np.zeros((3, 128, 128), dtype=np.float32)
        for c in range(3):
            wqd[c, :, 0:HD] = wq.T[c * 128:(c + 1) * 128]
            wqd[c, :, 64:64 + HD] = wq.T[c * 128:(c + 1) * 128]
            wkd[c, :, 0:HD] = wk.T[c * 128:(c + 1) * 128]
            wkd[c, :, 64:64 + HD] = wk.T[c * 128:(c + 1) * 128]
        wvc = np.ascontiguousarray(wv.T).reshape(3, 128, HD)

        def dupvec(v):
            o = np.zeros((128, 1), dtype=np.float32)
            o[0:HD, 0] = v
            o[64:64 + HD, 0] = v
            return o

        bq = dupvec(qkv_b[sl] * qs)
        bk = dupvec(qkv_b[DIM + h * HD:DIM + (h + 1) * HD])
        bv = dupvec(qkv_b[2 * DIM + h * HD:2 * DIM + (h + 1) * HD])
        lw = lepe_w[sl, 0].reshape(HD, 9)            # [48, 9] taps row-major
        dwt = np.zeros((9, 128, HD), dtype=np.float32)
        for i in range(9):
            dwt[i, 0:HD, :] = np.diag(lw[:, i])
            dwt[i, 64:64 + HD, :] = np.diag(lw[:, i])
        lb48 = np.zeros((128, HD), dtype=np.float32)
        lb48[0, :] = lepe_b[sl]
        lw9 = np.zeros((128, 9), dtype=np.float32)
        lw9[0:HD] = lw
        lw9[64:64 + HD] = lw

        in_maps.append({
            "xT": xT,
            "wqd": wqd.astype(bf16),
            "wkd": wkd.astype(bf16),
            "wv": wvc.astype(bf16),
            "bqd": bq, "bkd": bk, "bvd": bv,
            "dwt": dwt.astype(bf16), "lb48": lb48.astype(bf16), "lw9": lw9,
            "idn": idn,
        })
    return in_maps


def kernel(x, qkv_w, qkv_b, lepe_w, lepe_b, H=64, W=64):
    assert int(H) == 64 and int(W) == 64
    from concourse.bass_utils import run_bass_kernel_spmd

    if "nc" not in _CACHE:
        _CACHE["nc"] = _build_module()
    nc = _CACHE["nc"]

    in_maps = _prep_in_maps(x, qkv_w, qkv_b, lepe_w, lepe_b)
    res = run_bass_kernel_spmd(nc, in_maps, core_ids=list(range(NUM_HEADS)))

    full = np.empty((SEQ, DIM), dtype=np.float32)
    for h in range(NUM_HEADS):
        full[:, h * HD:(h + 1) * HD] = res.results[h]["out"].reshape(SEQ, HD)
    return full.reshape(B, N, DIM)


# revision 58
# speedup vs baseline: 1.0024x; 1.0024x over previous
"""Trainium2 Bass kernel for nn_Attention_13829794693777.

Multi-head attention (8 heads, head_dim 48) + LePE depthwise 3x3 conv on v.
Sharding: tensor-parallel over heads -- one head per NeuronCore (8 cores).
Each core gets the full (transposed, bf16) input activations plus its head's
qkv weight slice and LePE channel slice; it produces the [seq, 48] channel
slice of the output. The host concatenates slices and reshapes.

Device-side dataflow (per core / head):
  1. Projection (PE): qT/kT channel-major [128(dup), seq] bf16; q weights are
     pre-scaled by SCALE*C1 (C1 = 128/ln2) so raw scores arrive scaled for
     the Schraudolph-exp trick. v goes into a 66x66 zero-padded bf16 image
     (vT_pad). Bias copies q/k run on ACT (activation Identity+bias), the v
     scatter on DVE.
  2. v_aug [seq-tile, 49] bf16 via PE transposes reading the padded image
     directly; psum->SBUF copies batched 4-at-a-time on ACT.
  3. LePE: computed per chunk on PE as 9 shifted transpose-matmuls against
     diagonal tap matrices (+1 ones-row matmul for the bias), accumulating
     [pixel, ch] directly into the chunk's PSUM epilogue region.
  4. Attention per 512-query chunk, pipelined over a 3-deep pool of 2-bank
     PSUM scores tiles: S^T[k, q] matmuls (PE) -> exp (split ACT real-Exp /
     DVE Schraudolph int16-bitcast-bf16) -> PV matmuls out[q, ch] with P as
     stationary (output free dim = 49, not 512).
  5. Epilogue per chunk: reciprocal of the ones-column denominators + fused
     affine (out = attn*rec + lepe^T) on DVE, DMA out seq-major.
"""

import numpy as np
import ml_dtypes

NUM_HEADS = 8
DIM = 384
HD = 48
B = 2
N = 4096
SEQ = B * N          # 8192
IMG = 64             # H = W = 64
PADW = IMG + 2       # 66
PADN = PADW * PADW   # 4356
SCALE = HD ** -0.5
NCHUNK = SEQ // 512  # 16 query chunks of 512
KT_PER_B = N // 128  # 32 k-tiles per batch

C1 = float(128.0 / np.log(2.0))   # Schraudolph scale (baked into wq)
C2 = 16250.5                      # Schraudolph bias (RNE-calibrated)

# exp engine pattern per chunk: batches of 2 k-tiles.
# 'A' = ACT real exp, 'D' = DVE Schraudolph.
EXP_PATTERN = ['A', 'A', 'D', 'A', 'D', 'A', 'D', 'A',
               'D', 'A', 'D', 'A', 'D', 'A', 'D', 'A']
EXP_BW = 2
PV_LAG = 10
# LePE: taps with index in PE_TAPS stay on PE (diag matmuls); the rest are
# computed on DVE channel-major per image-eighth and transpose-accumulated.
PE_TAPS = [0, 4, 8]
LEPE_STEP = 6

_CACHE = {}


def _build_module():
    """Build (once) the Bacc module shared by all 8 cores."""
    import concourse.bacc as bacc
    import concourse.bass as bass
    import concourse.mybir as mybir
    import concourse.tile as tile
    from concourse.dve_ops import AFFINE_THEN_ADD
    from contextlib import ExitStack

    dt = mybir.dt
    AF = mybir.ActivationFunctionType
    ALU = mybir.AluOpType

    nc = bacc.Bacc("TRN2", target_bir_lowering=False, debug=False, num_devices=8)

    # ---- DRAM parameters -------------------------------------------------
    xT_d = nc.dram_tensor("xT", [3, 128, SEQ], dt.bfloat16, kind="ExternalInput").ap()
    wqd_d = nc.dram_tensor("wqd", [3, 128, 128], dt.bfloat16, kind="ExternalInput").ap()
    wkd_d = nc.dram_tensor("wkd", [3, 128, 128], dt.bfloat16, kind="ExternalInput").ap()
    wv_d = nc.dram_tensor("wv", [3, 128, HD], dt.bfloat16, kind="ExternalInput").ap()
    bqd_d = nc.dram_tensor("bqd", [128, 1], dt.float32, kind="ExternalInput").ap()
    bkd_d = nc.dram_tensor("bkd", [128, 1], dt.float32, kind="ExternalInput").ap()
    bvd_d = nc.dram_tensor("bvd", [128, 1], dt.float32, kind="ExternalInput").ap()
    dwt_d = nc.dram_tensor("dwt", [9, 128, HD], dt.bfloat16, kind="ExternalInput").ap()
    lb48_d = nc.dram_tensor("lb48", [128, HD], dt.bfloat16, kind="ExternalInput").ap()
    lw9_d = nc.dram_tensor("lw9", [128, 9], dt.float32, kind="ExternalInput").ap()
    idn_d = nc.dram_tensor("idn", [128, 64], dt.float32, kind="ExternalInput").ap()
    out_d = nc.dram_tensor("out", [64, 128, HD], dt.float32, kind="ExternalOutput").ap()
    out_v = out_d.rearrange("t p c -> p t c")

    with tile.TileContext(nc) as tc, ExitStack() as ctx:
        per = ctx.enter_context(tc.tile_pool(name="per", bufs=1))
        rot = ctx.enter_context(tc.tile_pool(name="rot", bufs=2))
        ptp = ctx.enter_context(tc.tile_pool(name="ptp", bufs=8))

        # ---- persistent SBUF tensors ------------------------------------
        xs = [[per.tile([128, 2048], dt.bfloat16, name=f"x{c}_{j}", tag=f"x{c}_{j}") for j in range(4)]
              for c in range(3)]
        qTd = [per.tile([128, 512], dt.bfloat16, name=f"q{n}", tag=f"q{n}") for n in range(NCHUNK)]
        kTd = [per.tile([128, 512], dt.bfloat16, name=f"k{n}", tag=f"k{n}") for n in range(NCHUNK)]
        v_aug = per.tile([128, 64 * 49], dt.bfloat16, name="vaug", tag="vaug")
        v_aug3 = v_aug[:].rearrange("p (t c) -> p t c", c=49)
        vT_pad = per.tile([128, PADN], dt.bfloat16, name="vpad", tag="vpad")
        vp3 = vT_pad[:].rearrange("p (a b) -> p a b", b=PADW)

        wq_all = per.tile([128, 384], dt.bfloat16, name="wqa", tag="wqa")
        wk_all = per.tile([128, 384], dt.bfloat16, name="wka", tag="wka")
        wv_all = per.tile([128, 3 * HD], dt.bfloat16, name="wva", tag="wva")
        wq_sb = [wq_all[:, 128 * c:128 * c + 128] for c in range(3)]
        wk_sb = [wk_all[:, 128 * c:128 * c + 128] for c in range(3)]
        wv_sb = [wv_all[:, HD * c:HD * c + HD] for c in range(3)]
        bq_sb = per.tile([128, 1], dt.float32, name="bq", tag="bq")
        bk_sb = per.tile([128, 1], dt.float32, name="bk", tag="bk")
        bv_sb = per.tile([128, 1], dt.float32, name="bv", tag="bv")
        acc_e = [per.tile([128, 512], dt.float32, name=f"acc{e}", tag=f"acc{e}")
                 for e in range(8)]
        dw_all = per.tile([128, 9 * HD], dt.bfloat16, name="dwa", tag="dwa")
        dw_sb = [dw_all[:, HD * i:HD * i + HD] for i in range(9)]
        lb_sb = per.tile([128, HD], dt.bfloat16, name="lb48", tag="lb48")
        dw9_sb = per.tile([128, 9], dt.float32, name="dw9", tag="dw9")
        one_sb = per.tile([128, 128], dt.bfloat16, name="ones", tag="ones")
        id_sb = per.tile([128, 64], dt.float32, name="idn", tag="idn")
        idb_sb = per.tile([128, 64], dt.bfloat16, name="idnb", tag="idnb")

        # ---- input DMAs: weights (1 DMA each) on the ACT queue, x split --
        nc.scalar.dma_start(wq_all[:].rearrange("p (c k) -> p c k", c=3),
                            wqd_d.rearrange("c p k -> p c k"))
        nc.scalar.dma_start(wk_all[:].rearrange("p (c k) -> p c k", c=3),
                            wkd_d.rearrange("c p k -> p c k"))
        nc.scalar.dma_start(wv_all[:].rearrange("p (c k) -> p c k", c=3),
                            wv_d.rearrange("c p k -> p c k"))
        nc.scalar.dma_start(bq_sb[:], bqd_d[:])
        nc.scalar.dma_start(bk_sb[:], bkd_d[:])
        nc.scalar.dma_start(bv_sb[:], bvd_d[:])
        for c in range(3):
            eng = nc.sync if c != 1 else nc.scalar
            eng.dma_start(xs[c][0][:, 0:512], xT_d[c, :, 0:512])
        for j in range(4):
            for c in range(3):
                eng = nc.sync if c != 1 else nc.scalar
                if j == 0:
                    eng.dma_start(xs[c][0][:, 512:2048], xT_d[c, :, 512:2048])
                else:
                    eng.dma_start(xs[c][j][:], xT_d[c, :, j * 2048:(j + 1) * 2048])
        nc.scalar.dma_start(dw_all[:].rearrange("p (t c) -> p t c", c=HD),
                            dwt_d.rearrange("t p c -> p t c"))
        nc.scalar.dma_start(lb_sb[:], lb48_d[:])
        nc.scalar.dma_start(dw9_sb[:], lw9_d[:])
        nc.vector.memset(one_sb[0:1, :], 1.0)
        nc.scalar.dma_start(id_sb[:], idn_d[:])
        nc.vector.tensor_copy(idb_sb[:], id_sb[:])

        # zero the padded image (borders must be 0)
        nc.vector.memset(vT_pad[:], 0.0)
        nc.vector.memset(v_aug3[:, :, 48:49], 1.0)

        taps = [(dr, dc) for dr in (-1, 0, 1) for dc in (-1, 0, 1)]
        dve_taps = [i for i in range(9) if i not in PE_TAPS]

        def emit_dve_lepe(e):
            # channel-major taps over image-eighth e (rows 8e..8e+8, both batches)
            def tap(dr, dc):
                return vp3[0:112, 1 + 8 * e + dr:1 + 8 * e + 8 + dr,
                           1 + dc:1 + dc + IMG]
            first = True
            for i in dve_taps:
                dr, dc = taps[i]
                if first:
                    nc.vector.tensor_scalar(acc_e[e][0:112, :], tap(dr, dc),
                                            dw9_sb[0:112, i:i + 1], None,
                                            op0=ALU.mult)
                    first = False
                else:
                    nc.vector.scalar_tensor_tensor(acc_e[e][0:112, :], tap(dr, dc),
                                                   dw9_sb[0:112, i:i + 1],
                                                   acc_e[e][0:112, :],
                                                   op0=ALU.mult, op1=ALU.add)

        # ---- phase 1: projection ----------------------------------------
        psA_ctx = ExitStack()
        psA = psA_ctx.enter_context(tc.tile_pool(name="psA", bufs=2, space="PSUM"))
        for n in range(NCHUNK):
            rhs = [xs[c][n // 4][:, (n % 4) * 512:(n % 4 + 1) * 512] for c in range(3)]
            b = n // 8
            rb = 64 * b
            pvv = psA.tile([128, 512], dt.float32, name="pvv", tag="pvv")
            for c in range(3):
                nc.tensor.matmul(pvv[rb:rb + HD, :], wv_sb[c][:], rhs[c],
                                 start=(c == 0), stop=(c == 2),
                                 tile_position=(0, rb))
            # scatter the 512 pixels (8 image rows) into the padded image
            r0 = 8 * (n % 8)
            dest = vp3[rb:rb + HD, 1 + r0:1 + r0 + 8, 1:65]
            nc.vector.tensor_scalar(dest, pvv[rb:rb + HD, :], bv_sb[rb:rb + HD, 0:1],
                                    None, op0=ALU.add)
            pq = psA.tile([128, 512], dt.float32, name="pq", tag="pq")
            for c in range(3):
                nc.tensor.matmul(pq[:], wq_sb[c][:], rhs[c],
                                 start=(c == 0), stop=(c == 2))
            nc.scalar.activation(qTd[n][:], pq[:], AF.Identity, bias=bq_sb[:, 0:1])
            pk = psA.tile([128, 512], dt.float32, name="pk", tag="pk")
            for c in range(3):
                nc.tensor.matmul(pk[:], wk_sb[c][:], rhs[c],
                                 start=(c == 0), stop=(c == 2))
            nc.scalar.activation(kTd[n][:], pk[:], AF.Identity, bias=bk_sb[:, 0:1])

            if n == 13:
                emit_dve_lepe(0)
            elif n == 14:
                emit_dve_lepe(1)

        psA_ctx.close()

        # ---- phase 2: v_aug via PE transposes from the padded image -----
        ps2_ctx = ExitStack()
        ps2 = ps2_ctx.enter_context(tc.tile_pool(name="ps2", bufs=6, space="PSUM"))
        for g in range(16):
            tq = ps2.tile([128, 192], dt.bfloat16, name="tq", tag="tq")
            for j in range(4):
                t = 4 * g + j
                b = t // 32
                rb = 64 * b
                tt = t % 32
                for h in range(2):
                    nc.tensor.matmul(tq[64 * h:64 * h + 64, 48 * j:48 * j + 48],
                                     vp3[rb:rb + HD, 1 + 2 * tt + h, 1:65],
                                     idb_sb[rb:rb + HD, 0:HD],
                                     is_transpose=True, tile_position=(rb, 64 * h))
            tq3 = tq[:].rearrange("p (t c) -> p t c", c=48)
            if g % 3 != 2:
                nc.scalar.activation(v_aug3[:, 4 * g:4 * g + 4, 0:48], tq3[:],
                                     AF.Copy)
            else:
                nc.vector.tensor_copy(v_aug3[:, 4 * g:4 * g + 4, 0:48], tq3[:])
        ps2_ctx.close()

        # ---- phase 4: main attention loop -------------------------------
        stp = ctx.enter_context(tc.tile_pool(name="stp", bufs=3, space="PSUM"))
        psv = ctx.enter_context(tc.tile_pool(name="psv", bufs=2, space="PSUM"))

        # batch layout: slot-aligned batches of EXP_BW k-tiles (slot = kt % 6)
        nb = KT_PER_B // EXP_BW
        batches = [(EXP_BW * i, EXP_BW * i + EXP_BW) for i in range(nb)]

        for cc in range(NCHUNK):
            bc = cc // 8
            rb = 64 * bc
            if cc < 6:
                emit_dve_lepe(cc + 2)
            pvb = psv.tile([128, 512], dt.float32, name="pvb", tag="pvb")
            pv3 = pvb[:, 0:196].rearrange("p (t c) -> p t c", c=49)

            # LePE [pixel, ch] for the 4 query tiles (cols 256..448):
            # 9 shifted diag-matmuls + 1 ones-row bias matmul, PSUM-accumulated
            # (emitted via emit_lepe after the first scores of the chunk so a
            # pvb-buffer wait can't head-of-line-block the scores stream)
            def emit_lepe(qs):
                # transpose of the DVE-computed taps comes FIRST: qs==0 carries
                # the bank 'start' (zeroes the whole pvb bank); everything else
                # (PE taps, bias, PV) accumulates after it.
                tt = (4 * cc + qs) % 32
                e8 = tt // 4
                nc.tensor.matmul(pvb[:, 256 + 48 * qs:256 + 48 * qs + 48],
                                 acc_e[e8][rb:rb + HD,
                                           128 * (tt % 4):128 * (tt % 4) + 128],
                                 id_sb[rb:rb + HD, 0:HD],
                                 is_transpose=True, tile_position=(rb, 0),
                                 start=(qs == 0), stop=False,
                                 skip_group_check=True)

            def emit_lepe_taps(qs):
                tt = (4 * cc + qs) % 32
                for h in range(2):
                    dst = pvb[64 * h:64 * h + 64, 256 + 48 * qs:256 + 48 * qs + 48]
                    for ii, i in enumerate(PE_TAPS):
                        dr, dc = taps[i]
                        nc.tensor.matmul(dst,
                                         vp3[rb:rb + HD, 1 + 2 * tt + h + dr,
                                             1 + dc:1 + dc + IMG],
                                         dw_sb[i][rb:rb + HD, :],
                                         start=False, stop=False,
                                         tile_position=(rb, 64 * h),
                                         skip_group_check=True)
                    nc.tensor.matmul(dst, one_sb[0:1, 0:64], lb_sb[0:1, :],
                                     start=False, stop=False,
                                     tile_position=(0, 64 * h),
                                     skip_group_check=True)

            pt_of_batch = {}
            bi = 0  # next batch whose scores are fully issued
            st_of_batch = {}

            def emit_exp(bidx):
                w = 512 * EXP_BW
                st = st_of_batch.pop(bidx)
                pt = ptp.tile([128, 512 * EXP_BW], dt.bfloat16, name="pt", tag="pt")
                if EXP_PATTERN[bidx] == 'A':
                    nc.scalar.activation(pt[:, 0:w], st[:, 0:w],
                                         AF.Exp, scale=float(1.0 / C1))
                else:
                    nc.vector.tensor_scalar(
                        pt[:, 0:w].bitcast(mybir.dt.int16),
                        st[:, 0:w], C2, None, op0=ALU.add)
                pt_of_batch[bidx] = pt

            def emit_pv(kt):
                bidx = kt // EXP_BW
                a, _ = batches[bidx]
                pt = pt_of_batch[bidx]
                for qb in range(4):
                    nc.tensor.matmul(pvb[:, 49 * qb:49 * qb + 49],
                                     pt[:, 512 * (kt - a) + 128 * qb:
                                         512 * (kt - a) + 128 * qb + 128],
                                     v_aug3[:, bc * 32 + kt, 0:49],
                                     start=False,
                                     stop=(kt == KT_PER_B - 1 and qb == 3),
                                     tile_position=(0, 0), skip_group_check=True)

            for step in range(KT_PER_B + PV_LAG):
                if step == LEPE_STEP:
                    for qs in range(4):
                        emit_lepe(qs)
                    for qs in range(4):
                        emit_lepe_taps(qs)
                kt = step
                if kt < KT_PER_B:
                    bidx = kt // EXP_BW
                    if kt % EXP_BW == 0:
                        st_of_batch[bidx] = stp.tile([128, 512 * EXP_BW],
                                                     dt.float32, name="st", tag="st")
                    row = 64 * (kt & 1)
                    ktile = kTd[bc * 8 + kt // 4]
                    koff = (kt % 4) * 128
                    j = kt % EXP_BW
                    nc.tensor.matmul(st_of_batch[bidx][:, 512 * j:512 * j + 512],
                                     ktile[row:row + HD, koff:koff + 128],
                                     qTd[cc][row:row + HD, :],
                                     tile_position=(row, 0))
                    if bi < len(batches) and kt + 1 == batches[bi][1]:
                        emit_exp(bi)
                        bi += 1
                pvkt = step - PV_LAG
                if 0 <= pvkt < KT_PER_B:
                    emit_pv(pvkt)

            # ---- epilogue for this 512-query chunk ----------------------
            tmp = rot.tile([128, 196], dt.float32, name="tmp", tag="tmp")
            rec = rot.tile([128, 4], dt.float32, name="rec", tag="rec")
            ot = rot.tile([128, 192], dt.float32, name="ot", tag="ot")
            nc.vector.tensor_copy(tmp[:], pvb[:, 0:196])
            tmp3 = tmp[:].rearrange("p (t c) -> p t c", c=49)
            nc.vector.reciprocal(rec[:], tmp3[:, :, 48:49])
            for qs in range(4):
                nc.vector._custom_dve(AFFINE_THEN_ADD,
                                      out=ot[:, qs * 48:(qs + 1) * 48],
                                      in0=tmp[:, qs * 49:qs * 49 + 48],
                                      in1=pvb[:, 256 + 48 * qs:256 + 48 * qs + 48],
                                      s0=rec[:, qs:qs + 1], s1=0.0)
            nc.sync.dma_start(out_v[:, 4 * cc:4 * cc + 4, :],
                              ot[:].rearrange("p (t c) -> p t c", c=HD))

    nc.compile()
    return nc


def _prep_in_maps(x, qkv_w, qkv_b, lepe_w, lepe_b):
    bf16 = ml_dtypes.bfloat16
    X = np.asarray(x, dtype=np.float32).reshape(SEQ, DIM)
    xT = np.ascontiguousarray(X.T).astype(bf16).reshape(3, 128, SEQ)

    qkv_w = np.asarray(qkv_w, dtype=np.float32)
    qkv_b = np.asarray(qkv_b, dtype=np.float32)
    lepe_w = np.asarray(lepe_w, dtype=np.float32)
    lepe_b = np.asarray(lepe_b, dtype=np.float32)

    idn = np.zeros((128, 64), dtype=np.float32)
    idn[0:64, 0:64] = np.eye(64, dtype=np.float32)
    idn[64:128, 0:64] = np.eye(64, dtype=np.float32)

    qs = SCALE * C1  # bake the Schraudolph scale into the q projection

    in_maps = []
    for h in range(NUM_HEADS):
        sl = slice(h * HD, (h + 1) * HD)
        wq = qkv_w[sl, :] * qs                       # [48, 384]
        wk = qkv_w[DIM + h * HD:DIM + (h + 1) * HD, :]
        wv = qkv_w[2 * DIM + h * HD:2 * DIM + (h + 1) * HD, :]
        wqd = np.zeros((3, 128, 128), dtype=np.float32)
        wkd = np.zeros((3, 128, 128), dtype=np.float32)
        for c in range(3):
            wqd[c, :, 0:HD] = wq.T[c * 128:(c + 1) * 128]
            wqd[c, :, 64:64 + HD] = wq.T[c * 128:(c + 1) * 128]
            wkd[c, :, 0:HD] = wk.T[c * 128:(c + 1) * 128]
            wkd[c, :, 64:64 + HD] = wk.T[c * 128:(c + 1) * 128]
        wvc = np.ascontiguousarray(wv.T).reshape(3, 128, HD)

        def dupvec(v):
            o = np.zeros((128, 1), dtype=np.float32)
            o[0:HD, 0] = v
            o[64:64 + HD, 0] = v
            return o

        bq = dupvec(qkv_b[sl] * qs)
        bk = dupvec(qkv_b[DIM + h * HD:DIM + (h + 1) * HD])
        bv = dupvec(qkv_b[2 * DIM + h * HD:2 * DIM + (h + 1) * HD])
        lw = lepe_w[sl, 0].reshape(HD, 9)            # [48, 9] taps row-major
        dwt = np.zeros((9, 128, HD), dtype=np.float32)
        for i in range(9):
            dwt[i, 0:HD, :] = np.diag(lw[:, i])
            dwt[i, 64:64 + HD, :] = np.diag(lw[:, i])
        lb48 = np.zeros((128, HD), dtype=np.float32)
        lb48[0, :] = lepe_b[sl]
        lw9 = np.zeros((128, 9), dtype=np.float32)
        lw9[0:HD] = lw
        lw9[64:64 + HD] = lw

        in_maps.append({
            "xT": xT,
            "wqd": wqd.astype(bf16),
            "wkd": wkd.astype(bf16),
            "wv": wvc.astype(bf16),
            "bqd": bq, "bkd": bk, "bvd": bv,
            "dwt": dwt.astype(bf16), "lb48": lb48.astype(bf16), "lw9": lw9,
            "idn": idn,
        })
    return in_maps


def kernel(x, qkv_w, qkv_b, lepe_w, lepe_b, H=64, W=64):
    assert int(H) == 64 and int(W) == 64
    from concourse.bass_utils import run_bass_kernel_spmd

    if "nc" not in _CACHE:
        _CACHE["nc"] = _build_module()
    nc = _CACHE["nc"]

    in_maps = _prep_in_maps(x, qkv_w, qkv_b, lepe_w, lepe_b)
    res = run_bass_kernel_spmd(nc, in_maps, core_ids=list(range(NUM_HEADS)))

    full = np.empty((SEQ, DIM), dtype=np.float32)
    for h in range(NUM_HEADS):
        full[:, h * HD:(h + 1) * HD] = res.results[h]["out"].reshape(SEQ, HD)
    return full.reshape(B, N, DIM)
